# revision 39
# baseline (speedup 1.0000x reference)
"""Trainium2 Bass kernel for nn_NodeModel (GNN message passing), bf16.

Reference computation:
    h   = relu(concat(x[row], edge_attr) @ W1 + b1) @ W2 + b2     # edge MLP
    agg = scatter_mean(h, col, N)                                  # per-dest mean
    out = relu(concat(x, agg) @ W3 + b3) @ W4 + b4                 # node MLP

Key algebraic restructure: scatter_mean is linear, so W2 commutes with it:
    agg = scatter_mean(relu(h1), col) @ W2 + b2*[deg>0]
which applies the 1280x1280 W2 matmul per NODE (~6.3k rows/core) instead
of per EDGE (~16k rows/core) — a ~2.5x FLOP cut on the largest matmul.

Distribution strategy (8 cores, no collectives):
  - Sort edges by destination node; split destination nodes into 8
    block-aligned, edge-balanced shards.  Each core owns one node shard and
    ALL edges targeting it, so per-node means are complete locally.
  - The gathered-and-transposed edge input concat(x[row], edge_attr)^T is
    prepared host-side per core, so the device runs no transposes at all:
      * h1 rows [edge, 1280] are computed directly (inT chunk slices
        stationary, W1 moving), relu'd and pre-scaled by 1/deg(dest) at
        PSUM drain, staged to DRAM,
      * scatter-mean gathers the rows of each 128-node destination block
        and matmuls h1_slice^T @ onehot(S), accumulating agg^T directly,
      * aggT = W2^T @ aggH1T per superblock (W2 slices stationary),
      * node MLP consumes agg^T and x^T and emits output rows directly.
  - All matmuls run in bfloat16 (1 cycle/row on the PE, like fp32r, but
    half the SBUF/DMA traffic, no fp32r small-free-dim penalty, and no
    transpose/HAM-throttle); accumulation is fp32 in PSUM.  fp8 was
    measured numerically out of reach for the 2e-2 gate.
  - PSUM accumulation groups are kept bank-safe: the 10 per-block scatter
    groups (512B each, sub-bank) are zero-initialized by three K=1
    bank-covering matmuls, and every real matmul accumulates with
    start=False (start_tensor_calc zeroes whole 2KB banks and would
    corrupt bank neighbours).
"""

import math
import sys
from contextlib import ExitStack

sys.path.insert(0, "/opt/trn_rl_repo")

import numpy as np
import ml_dtypes

import concourse.bass as bass
import concourse.tile as tile
from concourse import bacc, mybir
from concourse.bass_utils import run_bass_kernel_spmd

NCORES = 8
P = 128
FN = 512    # node feature dim
FE = 128    # edge feature dim
HID = 1280  # edge-MLP hidden/output dim
IN1 = FN + FE          # 640
IN2 = FN + HID         # 1792
BF16 = mybir.dt.bfloat16
F32 = mybir.dt.float32
I32 = mybir.dt.int32
RELU = mybir.ActivationFunctionType.Relu
COPY = mybir.ActivationFunctionType.Copy
NPBF = ml_dtypes.bfloat16

_prog_cache = {}


def _build(EC, NB, KB, use_b1, use_b2, use_b4):
    """Build the SPMD program for one core.

    EC: edge chunks (128 edges each) per core, multiple of 4.
    NB: node blocks (128 nodes each) per core, multiple of 4.
    KB: max edge chunks per node block (scatter schedule width).
    """
    EP = EC * P
    NBP = NB * P
    SC = EC // 4         # superchunks of 512 edges
    NSB = (NB + 3) // 4  # superblocks of <=512 nodes (last may be partial)

    nc = bacc.Bacc("TRN2", target_bir_lowering=False, debug=False,
                   num_devices=NCORES)

    inT_d = nc.dram_tensor("inT", [P, 5, EP], BF16, kind="ExternalInput")
    W1_d = nc.dram_tensor("W1", [IN1, HID], BF16, kind="ExternalInput")
    W2_d = nc.dram_tensor("W2", [HID, HID], BF16, kind="ExternalInput")
    W3_d = nc.dram_tensor("W3", [IN2, IN1], BF16, kind="ExternalInput")
    W4_d = nc.dram_tensor("W4", [IN1, FN], BF16, kind="ExternalInput")
    b3_d = nc.dram_tensor("b3", [P, IN1 // P], F32, kind="ExternalInput")
    gid_d = nc.dram_tensor("gid", [P, NB * KB], I32, kind="ExternalInput")
    colb_d = nc.dram_tensor("colb", [P, NB * KB], F32, kind="ExternalInput")
    invce_d = nc.dram_tensor("invce", [P, EC], F32, kind="ExternalInput")
    xsT_d = nc.dram_tensor("xsT", [P, 4, NBP], BF16, kind="ExternalInput")
    iota_d = nc.dram_tensor("iota", [P, P], F32, kind="ExternalInput")
    if use_b1:
        b1r_d = nc.dram_tensor("b1r", [P, HID], F32, kind="ExternalInput")
    if use_b2:
        b2r_d = nc.dram_tensor("b2r", [1, HID], BF16, kind="ExternalInput")
        m2r_d = nc.dram_tensor("m2r", [1, NBP], BF16, kind="ExternalInput")
    if use_b4:
        b4r_d = nc.dram_tensor("b4r", [1, FN], BF16, kind="ExternalInput")
        onesr_d = nc.dram_tensor("onesr", [1, P], BF16, kind="ExternalInput")
    out_d = nc.dram_tensor("out", [NBP, FN], F32, kind="ExternalOutput")
    h1_d = nc.dram_tensor("h1buf", [EP, HID], BF16)  # internal staging

    with tile.TileContext(nc) as tc, ExitStack() as ctx:
        cpool = ctx.enter_context(tc.tile_pool(name="const", bufs=1))

        # The first h1 matmul needs only W1 + the first inT slice; issue
        # everything else after them so the PE doesn't wait out ~8MB of
        # weight DMA at startup.
        W1t = cpool.tile([P, 5, HID], BF16)
        W1r = W1_d.ap().rearrange("(ko ki) m -> ki ko m", ki=P)
        for k in range(5):
            nc.scalar.dma_start(W1t[:, k, :], W1r[:, k, :])
        invct = cpool.tile([P, EC], F32)
        nc.scalar.dma_start(invct[:], invce_d.ap()[:])

        # ------------- pools: single scope, PSUM shared E/S -------------
        if True:
            inp = ctx.enter_context(tc.tile_pool(name="inT", bufs=2))
            h1p = ctx.enter_context(tc.tile_pool(name="h1r", bufs=3))
            tbp = (ctx.enter_context(tc.tile_pool(name="tb", bufs=2))
                   if use_b1 else None)
            h1gp = ctx.enter_context(tc.tile_pool(name="h1g", bufs=5 * KB))
            Sp = ctx.enter_context(tc.tile_pool(name="Smat", bufs=5 * KB))
            agHp = ctx.enter_context(tc.tile_pool(name="agH", bufs=3))
            aggp = ctx.enter_context(tc.tile_pool(name="aggT", bufs=2))
            xsp = ctx.enter_context(tc.tile_pool(name="xs", bufs=4))
            h3p = ctx.enter_context(tc.tile_pool(name="h3T", bufs=2))
            ogp = ctx.enter_context(tc.tile_pool(name="og", bufs=2))
            mmp = ctx.enter_context(
                tc.tile_pool(name="mm", bufs=2, space="PSUM"))
            mmp2 = mmp
            spp = ctx.enter_context(
                tc.tile_pool(name="spp", bufs=2, space="PSUM"))
            pend_gs = {}

            def gather_S(b):
                lst = []
                for k in range(KB):
                    c = b * KB + k
                    h1g = h1gp.tile([P, HID], BF16, name=f"h1g_{b}_{k}",
                                    tag="h1g")
                    nc.gpsimd.indirect_dma_start(
                        out=h1g[:], out_offset=None, in_=h1_d.ap()[:],
                        in_offset=bass.IndirectOffsetOnAxis(
                            ap=gidt[:, c:c + 1], axis=0),
                        bounds_check=EP - 1, oob_is_err=False)
                    St = Sp.tile([P, P], BF16, name=f"S_{b}_{k}", tag="S")
                    nc.vector.tensor_tensor(
                        St[:], colbt[:, c:c + 1].to_broadcast([P, P]),
                        iotat[:], op=mybir.AluOpType.is_equal)
                    lst.append((h1g, St))
                pend_gs[b] = lst

            def load_xst(s):
                w = min(512, NBP - s * 512)
                t = xsp.tile([P, 4, w], BF16, name=f"xst_{s}", tag="xst")
                nc.scalar.dma_start(
                    t[:], xsT_d.ap()[:, :, s * 512:s * 512 + w])
                return t

            def alloc_sp():
                # 10 sub-bank accumulation groups: zero the tile on the
                # DVE, then accumulate every matmul with start=False
                # (start_tensor_calc zeroes whole 2KB banks and would
                # corrupt bank neighbours).
                sp = spp.tile([P, 10 * P], F32, name="sp", tag="sp")
                nc.vector.memset(sp[:], 0.0)
                return sp

            def load_in(sc):
                t = inp.tile([P, 5, 512], BF16, name=f"in_{sc}", tag="inT")
                nc.sync.dma_start(t[:], inT_d.ap()[:, :, sc * 512:(sc + 1) * 512])
                return t

            in_cur = load_in(0)

            iotat = cpool.tile([P, P], F32)
            nc.scalar.dma_start(iotat[:], iota_d.ap()[:])
            b3t = cpool.tile([P, IN1 // P], F32)
            nc.scalar.dma_start(b3t[:], b3_d.ap()[:])
            gidt = cpool.tile([P, NB * KB], I32)
            nc.scalar.dma_start(gidt[:], gid_d.ap()[:])
            colbt = cpool.tile([P, NB * KB], F32)
            nc.scalar.dma_start(colbt[:], colb_d.ap()[:])
            if use_b1:
                b1rt = cpool.tile([P, HID], F32)
                nc.scalar.dma_start(b1rt[:], b1r_d.ap()[:])
            # The phase-S weights are streamed in behind the first few inT
            # slices (issued inside the sc loop) so the inT double-buffer
            # never queues behind ~8MB of weight DMA.
            W2t = cpool.tile([P, 10, HID], BF16)
            W3t = cpool.tile([P, 14, IN1], BF16)
            W4t = cpool.tile([P, 5, FN], BF16)
            b2rt = m2rt = b4rt = onesrt = None
            if use_b2:
                b2rt = cpool.tile([1, HID], BF16)
                m2rt = cpool.tile([1, NBP], BF16)
            if use_b4:
                b4rt = cpool.tile([1, FN], BF16)
                onesrt = cpool.tile([1, P], BF16)

            def load_phase_s_weights(sc):
                if sc == 0:
                    W2r = W2_d.ap().rearrange("(ko ki) m -> ki ko m", ki=P)
                    for k in range(10):
                        nc.scalar.dma_start(W2t[:, k, :], W2r[:, k, :])
                elif sc == 1:
                    nc.scalar.dma_start(
                        W3t[:],
                        W3_d.ap().rearrange("(ko ki) m -> ki ko m", ki=P))
                elif sc == 2:
                    nc.scalar.dma_start(
                        W4t[:],
                        W4_d.ap().rearrange("(ko ki) m -> ki ko m", ki=P))
                    if use_b2:
                        nc.scalar.dma_start(b2rt[:], b2r_d.ap()[:])
                        nc.scalar.dma_start(m2rt[:], m2r_d.ap()[:])
                    if use_b4:
                        nc.scalar.dma_start(b4rt[:], b4r_d.ap()[:])
                        nc.scalar.dma_start(onesrt[:], onesr_d.ap()[:])
            def edge_sc(sc, in_cur):
                in_next = load_in(sc + 1) if sc + 1 < SC else None
                load_phase_s_weights(sc)
                # h1 rows [e, hid1]: stationary inT chunk slices, moving W1
                for c in range(4):
                    ci = sc * 4 + c
                    h1t = h1p.tile([P, HID], BF16, name=f"h1_{sc}_{c}",
                                   tag="h1r")
                    for g in range(3):
                        lo = g * 512
                        hi = min(lo + 512, HID)
                        ps = mmp.tile([P, hi - lo], F32)
                        for k in range(5):
                            nc.tensor.matmul(
                                ps[:], in_cur[:, k, c * P:(c + 1) * P],
                                W1t[:, k, lo:hi], start=(k == 0), stop=(k == 4))
                        if use_b1:
                            tb = tbp.tile([P, hi - lo], F32)
                            nc.vector.tensor_tensor(
                                tb[:], ps[:], b1rt[:, lo:hi],
                                op=mybir.AluOpType.add)
                            nc.scalar.activation(
                                h1t[:, lo:hi], tb[:], RELU,
                                scale=invct[:, ci:ci + 1])
                        else:
                            # max(psum*invc, 0) = relu(psum)/deg on DVE
                            nc.vector.tensor_scalar(
                                h1t[:, lo:hi], ps[:],
                                invct[:, ci:ci + 1], 0.0,
                                op0=mybir.AluOpType.mult,
                                op1=mybir.AluOpType.max)
                    r0 = ci * P
                    nc.sync.dma_start(h1_d.ap()[r0:r0 + P, :], h1t[:])
                return in_next

            for sc in range(SC):
                in_cur = edge_sc(sc, in_cur)

        # ---------- Phases S+N: scatter-mean, W2, node MLP ----------
        if True:
            for b0 in range(min(4, NB)):
                gather_S(b0)
            xst_cur = load_xst(0)
            sp_cur = alloc_sp()

            def node_stage(s, agHt, xst, w):
                # aggT [hid2, n] = W2^T @ aggH1T (+ b2*[deg>0])
                aggTt = aggp.tile([P, 10, 512], BF16)
                for of in range(10):
                    ps = mmp2.tile([P, w], F32)
                    for k in range(10):
                        nc.tensor.matmul(
                            ps[:], W2t[:, k, of * P:(of + 1) * P],
                            agHt[:, k, 0:w], start=(k == 0),
                            stop=(k == 9 and not use_b2))
                    if use_b2:
                        nc.tensor.matmul(
                            ps[:], b2rt[0:1, of * P:(of + 1) * P],
                            m2rt[0:1, s * 512:s * 512 + w],
                            start=False, stop=True)
                    nc.vector.tensor_copy(aggTt[:, of, 0:w], ps[:])
                # h3T [of, n]: stationary W3 slices, moving xsT/aggT
                h3Tt = h3p.tile([P, 5, 512], BF16)
                for of in range(5):
                    ps = mmp2.tile([P, w], F32)
                    for k in range(4):
                        nc.tensor.matmul(
                            ps[:], W3t[:, k, of * P:(of + 1) * P],
                            xst[:, k, :], start=(k == 0), stop=False)
                    for k in range(10):
                        nc.tensor.matmul(
                            ps[:], W3t[:, 4 + k, of * P:(of + 1) * P],
                            aggTt[:, k, 0:w], start=False, stop=(k == 9))
                    nc.scalar.activation(h3Tt[:, of, 0:w], ps[:], RELU,
                                         bias=b3t[:, of:of + 1])
                # out [n, feat] rows: stationary h3T slices, moving W4
                for c in range(w // P):
                    ps = mmp2.tile([P, FN], F32)
                    for k in range(5):
                        nc.tensor.matmul(
                            ps[:], h3Tt[:, k, c * P:(c + 1) * P],
                            W4t[:, k, :], start=(k == 0),
                            stop=(k == 4 and not use_b4))
                    if use_b4:
                        nc.tensor.matmul(
                            ps[:], onesrt[0:1, :], b4rt[0:1, :],
                            start=False, stop=True)
                    og = ogp.tile([P, FN], F32, name=f"og_{s}_{c}", tag="og")
                    nc.scalar.activation(og[:], ps[:], COPY)
                    r0 = s * 512 + c * P
                    nc.sync.dma_start(out_d.ap()[r0:r0 + P, :], og[:])

            # Software-pipelined by one superblock: the PE runs superblock
            # s's scatter while s-1's aggT/h3/out matmuls fill the gaps the
            # DVE copies would otherwise stall.  Gathers for superblock s+1
            # are issued before s-1's node stage so the slow SW-DGE works
            # through it.
            pend_node = []
            for s in range(NSB):
                T = min(4, NB - s * 4)
                agHt = agHp.tile([P, 10, 512], BF16)
                for bb in range(T):
                    b = s * 4 + bb
                    for k, (h1g, St) in enumerate(pend_gs.pop(b)):
                        for hs in range(10):
                            nc.tensor.matmul(
                                sp_cur[:, hs * P:(hs + 1) * P],
                                h1g[:, hs * P:(hs + 1) * P],
                                St[:], start=False, stop=(k == KB - 1))
                    sp_done = sp_cur
                    if b + 1 < NB:
                        sp_cur = alloc_sp()  # memset ahead of the copies
                    for hs in range(10):
                        nc.vector.tensor_copy(
                            agHt[:, hs, bb * P:(bb + 1) * P],
                            sp_done[:, hs * P:(hs + 1) * P])
                for bn in range(s * 4 + 4, min(s * 4 + 8, NB)):
                    gather_S(bn)
                if len(pend_node) == 2:
                    node_stage(*pend_node.pop(0))
                xst = xst_cur
                xst_cur = load_xst(s + 1) if s + 1 < NSB else None
                pend_node.append((s, agHt, xst, T * P))
            for args in pend_node:
                node_stage(*args)

    nc.compile()
    return nc


def _prepare(x8, row, col, ea8):
    """Host-side sharding: sort edges by destination, split nodes into 8
    block-aligned edge-balanced shards, build per-core arrays (bf16)."""
    N = x8.shape[0]
    E = ea8.shape[0]
    order = np.argsort(col, kind="stable")
    scol = col[order]
    srow = row[order]
    NBLK = (N + P - 1) // P
    NTOT = NBLK * P

    # Cost-balanced block partition: per-core PE cycles ~ 6400/edge-chunk
    # (h1) + ~29k/node-block (scatter + W2 + node MLP); EC and NB are the
    # global maxima, so minimize 6400*EC(maxE) + 29140*maxNB.
    e_blk = np.bincount((scol // P).astype(np.int64), minlength=NBLK)
    cume = np.concatenate([[0], np.cumsum(e_blk)])
    best = None
    cap0 = int(math.ceil(NBLK / NCORES))
    for cap in range(cap0, cap0 + 4):
        bl = [0]
        ok = True
        for p in range(1, NCORES):
            target = p * E / NCORES
            lo = max(bl[-1] + 1, NBLK - (NCORES - p) * cap)
            hi = min(bl[-1] + cap, NBLK - (NCORES - p))
            if lo > hi:
                ok = False
                break
            bs = np.arange(lo, hi + 1)
            bl.append(int(bs[np.argmin(np.abs(cume[bs] - target))]))
        if not ok:
            continue
        bl.append(NBLK)
        maxNB = max(bl[i + 1] - bl[i] for i in range(NCORES))
        maxE = max(cume[bl[i + 1]] - cume[bl[i]] for i in range(NCORES))
        ECc = max(4, ((math.ceil(maxE / P) + 3) // 4) * 4)
        cost = 6400 * ECc + 29140 * maxNB
        if best is None or cost < best[0]:
            best = (cost, [b * P for b in bl])
    bounds = best[1]
    for p in range(1, NCORES + 1):
        assert bounds[p] > bounds[p - 1], f"degenerate shard bounds {bounds}"

    e_split = np.searchsorted(scol, bounds)
    Ec = np.diff(e_split)
    EC = max(4, math.ceil(int(Ec.max()) / P))
    EC = ((EC + 3) // 4) * 4
    EP = EC * P
    nblk = [(bounds[p + 1] - bounds[p]) // P for p in range(NCORES)]
    NB = max(4, int(max(nblk)))
    NBP = NB * P
    blkdeg = np.bincount(scol // P, minlength=NBLK)
    KB = max(1, math.ceil(int(blkdeg.max()) / P))

    deg = np.bincount(scol, minlength=NTOT + NBP).astype(np.float32)
    inve_all = 1.0 / np.maximum(deg[scol], 1.0)  # per sorted edge

    xpadT = np.zeros((FN, NTOT + NBP), NPBF)
    xpadT[:, :N] = x8.T

    cores = []
    for p in range(NCORES):
        s, e = int(e_split[p]), int(e_split[p + 1])
        n0 = bounds[p]
        ne = e - s
        # gathered+transposed edge-MLP input [ki, ko, e]
        feat = np.zeros((EP, IN1), NPBF)
        feat[:ne, :FN] = x8[srow[s:e]]
        feat[:ne, FN:] = ea8[order[s:e]]
        inT = np.ascontiguousarray(
            feat.T.reshape(5, P, EP).transpose(1, 0, 2))
        # per-edge 1/deg(dest) in [ki, chunk] layout
        ive = np.zeros(EP, np.float32)
        ive[:ne] = inve_all[s:e]
        invce = np.ascontiguousarray(ive.reshape(EC, P).T)
        # scatter schedule
        lcol = (scol[s:e] - n0).astype(np.int64)
        bstart = np.searchsorted(lcol, np.arange(NB + 1) * P)
        gid = np.full((NB, KB, P), 1 << 30, np.int32)
        gid.reshape(NB * KB, P)[:3 * KB] = 0  # first tiles: finite data
        colb = np.full((NB, KB, P), -1.0, np.float32)
        for b in range(NB):
            sb, eb = int(bstart[b]), int(bstart[b + 1])
            cnt = eb - sb
            assert cnt <= KB * P
            gid[b].reshape(-1)[:cnt] = np.arange(sb, eb, dtype=np.int32)
            colb[b].reshape(-1)[:cnt] = (lcol[sb:eb] - b * P)
        gid_t = np.ascontiguousarray(gid.reshape(NB * KB, P).T)
        colb_t = np.ascontiguousarray(colb.reshape(NB * KB, P).T)
        xsT = np.ascontiguousarray(
            xpadT[:, n0:n0 + NBP].reshape(4, P, NBP).transpose(1, 0, 2))
        ndeg = deg[n0:n0 + NBP]
        cores.append(dict(inT=inT, invce=invce, gid=gid_t, colb=colb_t,
                          xsT=xsT, ndeg=ndeg))
    return cores, bounds, EC, NB, KB


def _run(inputs, trace=False):
    x = np.asarray(inputs["x"], dtype=np.float32)
    ei = np.asarray(inputs["edge_index"])
    ea = np.asarray(inputs["edge_attr"], dtype=np.float32)
    row = ei[0].astype(np.int64)
    col = ei[1].astype(np.int64)
    x8 = x.astype(NPBF)
    ea8 = ea.astype(NPBF)
    W1 = np.ascontiguousarray(np.asarray(inputs["W1"], np.float32)).astype(NPBF)
    W2 = np.ascontiguousarray(np.asarray(inputs["W2"], np.float32)).astype(NPBF)
    W3 = np.ascontiguousarray(np.asarray(inputs["W3"], np.float32)).astype(NPBF)
    W4 = np.ascontiguousarray(np.asarray(inputs["W4"], np.float32)).astype(NPBF)
    b1 = np.asarray(inputs["b1"], np.float32)
    b2 = np.asarray(inputs["b2"], np.float32)
    b3 = np.asarray(inputs["b3"], np.float32)
    b4 = np.asarray(inputs["b4"], np.float32)
    N = x.shape[0]

    cores, bounds, EC, NB, KB = _prepare(x8, row, col, ea8)
    use_b1 = bool(np.any(b1))
    use_b2 = bool(np.any(b2))
    use_b4 = bool(np.any(b4))

    key = (EC, NB, KB, use_b1, use_b2, use_b4)
    if key not in _prog_cache:
        _prog_cache[key] = _build(EC, NB, KB, use_b1, use_b2, use_b4)
    nc = _prog_cache[key]

    b3t = np.ascontiguousarray(b3.reshape(IN1 // P, P).T)
    iota = np.ascontiguousarray(
        np.broadcast_to(np.arange(P, dtype=np.float32), (P, P)))

    in_maps = []
    for p in range(NCORES):
        c = cores[p]
        m = {
            "inT": c["inT"], "W1": W1, "W2": W2, "W3": W3, "W4": W4,
            "b3": b3t, "gid": c["gid"], "colb": c["colb"],
            "invce": c["invce"], "xsT": c["xsT"], "iota": iota,
        }
        if use_b1:
            m["b1r"] = np.ascontiguousarray(
                np.broadcast_to(b1.reshape(1, HID), (P, HID))).astype(
                    np.float32)
        if use_b2:
            m["b2r"] = np.ascontiguousarray(b2.reshape(1, HID)).astype(NPBF)
            m["m2r"] = (c["ndeg"] > 0).reshape(1, -1).astype(NPBF)
        if use_b4:
            m["b4r"] = np.ascontiguousarray(b4.reshape(1, FN)).astype(NPBF)
            m["onesr"] = np.ones((1, P), NPBF)
        in_maps.append(m)

    res = run_bass_kernel_spmd(nc, in_maps, list(range(NCORES)), trace=trace)

    out = np.empty((N, FN), np.float32)
    for p in range(NCORES):
        n0, n1 = bounds[p], min(bounds[p + 1], N)
        if n1 > n0:
            out[n0:n1] = res.results[p]["out"][:n1 - n0]
    return out, res


def kernel(**inputs) -> np.ndarray:
    out, _ = _run(inputs, trace=False)
    return out
